# revision 69
# baseline (speedup 1.0000x reference)
"""AncProbsLayer Trainium2 kernel (8 NeuronCores, SPMD data-parallel over batch b).

Math: for each (m, b, k):  P = expm(tau[m,b] * Q[m,k])  (20x20 GTR rate matrix),
then anc[m,b,l,k,:] = P[m,b,k, seq[m,b,l], :].

Host does the O(m*k*S^3) eigensolve preprocessing of the 16 tiny 20x20
matrices (R/p/Q/B/eigh -> V, W, lam tables), plus pure index re-encodings
(one-hots of sequences / rate_indices) and softplus of the tiny (m,b)
tau_kernel.  The device computes everything that scales with b/L/k: the tau
gather, e=exp(tau*lam) (ACT), P = (V.e) @ W (PE), the one-hot gather matmul
(PE), and the 21MB output production + DMA.  b is sharded 8 ways.

Per-core layout:
  pairs pr = m*4+q (q = local b), halves h in {0,1} = k groups of 4.
  P-matmul: out[20i, 160kj] += Ve[80(k,s), 20i]^T @ W[80(k,s), 160kj].
  Pairs are packed two-per-tile at 32-partition offsets (PE base-partition
  rule allows bases {0,32,64}); group g = m*2 + q//2, u = q%2.
  Gather matmul (chunk ci): out[128l, 160kj] = oh[20i, 128l]^T @ Ptab,
  where chunk ci covers l = 4*p + ci (stride-4 interleave) so each pair's
  [128, 640] output tile maps to a fully-contiguous 320KB DRAM region.

DMA-instruction count and completion latency are the scarce resources
(~626ns serialized HWDGE + ~900ns completion-semaphore per DMA): constants
ride 4 packed input DMAs ordered by critical path (aux|lam, V|W, oh[g0],
oh[g1:]); output is 8 per-pair DMAs whose data phases run back-to-back on
the DMA engines (~7.3us = the 2.6MB/core write at ~360GB/s).  Gather
results land in [128, 512] one-bank PSUM tiles (6 in flight) drained by
one [128, 2x160] copy each on alternating DVE/ACT.
"""

import sys
import numpy as np

for _p in ("/opt/trn_rl_repo", "/root/.axon_site/_ro/trn_rl_repo"):
    if _p not in sys.path:
        sys.path.append(_p)

M, B, L, K, S = 2, 32, 512, 8, 20
NCORES = 8
BLOC = B // NCORES          # 4 b's per core
NPAIR = M * BLOC            # 8 (m, q) pairs per core
NGRP = NPAIR // 2           # 4 groups of 2 pairs
NH = 2                      # k halves
KH = K // NH                # 4 k per half
KD = KH * S                 # 80 = contraction dim per half
KJ = K * S                  # 160 = (k, j) output free dim
NCHUNK = 4                  # l interleave factor
LC = L // NCHUNK            # 128
EPS = 1e-16

# fp16 matmul operands: 1 cycle/row on PE (fp32 = 4) with 10 mantissa bits.
_MM_NP = "float16"
AUXW = M * KD + M * BLOC                     # 168 f32 cols
AUXL = AUXW + M * NH + M * NH * S // 2       # + 4 lam + 40 V-as-f32 = 212
CST_F = M * NH * KJ                          # 640 fp16 cols (W only)

_GRAPH_CACHE = {}


def _softplus(x):
    return np.log1p(np.exp(-np.abs(x))) + np.maximum(x, 0.0)


def _host_prep(sequences, rate_indices, tau_kernel, exchangeability_kernel,
               equilibrium_kernel):
    """Eigensolve preprocessing of the 16 20x20 kernels + input staging."""
    ex = np.asarray(exchangeability_kernel, np.float64)
    eq = np.asarray(equilibrium_kernel, np.float64)
    R = _softplus(0.5 * (ex + np.swapaxes(ex, -1, -2)))          # (m,k,S,S)
    z = eq - eq.max(-1, keepdims=True)
    p = np.exp(z)
    p /= p.sum(-1, keepdims=True)                                # (m,k,S)
    Q = R * p[..., None, :]
    Q = Q - Q.sum(-1, keepdims=True) * np.eye(S)
    mue = -np.sum(p * np.diagonal(Q, axis1=-2, axis2=-1), axis=-1, keepdims=True)
    Q = Q / np.maximum(mue, EPS)[..., None]
    sqrtp = np.sqrt(p)
    Bm = sqrtp[..., :, None] * Q / sqrtp[..., None, :]
    Bm = 0.5 * (Bm + np.swapaxes(Bm, -1, -2))
    lam, U = np.linalg.eigh(Bm)                                  # (m,k,S),(m,k,S,S)
    V = U / sqrtp[..., :, None]                                  # V[m,k,i,s]
    Wm = U * sqrtp[..., :, None]                                 # W[m,k,j,s]

    p_dt = np.dtype(_MM_NP)
    # All small constants packed into ONE [128, CST_F] fp16 tensor (one DMA,
    # one completion-semaphore edge): aux (f32-as-fp16 bytes, rows 0:32),
    # lam (f32 bytes), V, W (fp16, rows 0:KD).
    V_f = np.zeros((KD, M * NH, S), p_dt)
    W_f = np.zeros((KD, M * NH, KJ), p_dt)
    lam_f32 = np.zeros((KD, M * NH), np.float32)
    for m in range(M):
        for h in range(NH):
            mh = m * NH + h
            for kq in range(KH):
                k = h * KH + kq
                r0 = kq * S
                V_f[r0:r0 + S, mh, :] = V[m, k].T.astype(p_dt)
                W_f[r0:r0 + S, mh, k * S:(k + 1) * S] = Wm[m, k].T.astype(p_dt)
                lam_f32[r0:r0 + S, mh] = lam[m, k]

    sp_tauT = _softplus(np.asarray(tau_kernel, np.float64)).T.astype(np.float32)

    seq = np.asarray(sequences)
    ri = np.asarray(rate_indices)
    in_maps = []
    for c in range(NCORES):
        b0 = c * BLOC
        # one-hot of sequences: oh[32u+i, g, l] = (seq[m, b0+2*(g%2)+u, l]==i)
        oh = np.zeros((64, NGRP, L), p_dt)
        for g in range(NGRP):
            m, qh = g // 2, g % 2
            for u in range(2):
                sq = seq[m, b0 + 2 * qh + u]
                oh[32 * u + sq, g, np.arange(L)] = 1.0
        # aux[:, m*KD:(m+1)*KD] = softplus(tau_kernel)[m] replicated KD wide
        # (so one matmul yields tau broadcast over the 80 contraction rows);
        # aux[:, 2*KD + m*4 + q] = rate one-hot.
        aux = np.zeros((B, M * KD + M * BLOC), np.float32)
        for m in range(M):
            aux[:, m * KD:(m + 1) * KD] = sp_tauT[:, m:m + 1]
            for q in range(BLOC):
                aux[ri[m, b0 + q], M * KD + m * BLOC + q] = 1.0
        cst = np.ascontiguousarray(W_f.reshape(KD, M * NH * KJ))
        auxlam = np.zeros((KD, AUXL), np.float32)
        auxlam[0:B, 0:AUXW] = aux
        auxlam[:, AUXW:AUXW + M * NH] = lam_f32
        auxlam[:, AUXW + M * NH:] = \
            V_f.reshape(KD, M * NH * S).view(np.float32)
        in_maps.append({
            "oh": oh,
            "auxlam": auxlam,
            "cst": cst,
        })
    return in_maps


def _build_graph():
    if "nc" in _GRAPH_CACHE:
        return _GRAPH_CACHE["nc"]
    from contextlib import ExitStack
    import concourse.mybir as mybir
    import concourse.tile as tile
    from concourse import bacc

    f32 = mybir.dt.float32
    mm_dt = getattr(mybir.dt, _MM_NP)
    AF = mybir.ActivationFunctionType
    ALU = mybir.AluOpType

    nc = bacc.Bacc("TRN2", target_bir_lowering=False, debug=False,
                   enable_asserts=False)
    oh_e = nc.declare_dram_parameter("oh", [64, NGRP, L], mm_dt, isOutput=False)
    aux_e = nc.declare_dram_parameter("auxlam", [KD, AUXL], f32, isOutput=False)
    cst_e = nc.declare_dram_parameter("cst", [KD, CST_F], mm_dt, isOutput=False)
    out_e = nc.declare_dram_parameter("out", [NPAIR, LC, NCHUNK * KJ], f32,
                                      isOutput=True)

    with tile.TileContext(nc) as tc, ExitStack() as ctx:
        const = ctx.enter_context(tc.tile_pool(name="const", bufs=1))
        work = ctx.enter_context(tc.tile_pool(name="work", bufs=3))
        outp = ctx.enter_context(tc.tile_pool(name="outp", bufs=8))
        ps_p = ctx.enter_context(tc.tile_pool(name="ps_p", bufs=2, space="PSUM"))
        ps_g = ctx.enter_context(tc.tile_pool(name="ps_g", bufs=6, space="PSUM"))

        # ---- packed input DMAs, one ring, critical-path order: V|W first
        # (needed mid-chain), tiny aux|lam second (arrives earlier: its data
        # phase is ~30ns), oh[g0] third, the rest last.
        aux_t = const.tile([KD, AUXL], f32, tag="aux_t")
        nc.sync.dma_start(aux_t[:], aux_e[:])
        cst = const.tile([KD, CST_F], mm_dt, tag="cst")
        nc.sync.dma_start(cst[:], cst_e[:])
        aux = aux_t[0:B, 0:AUXW]
        lam_a = aux_t[:, AUXW:AUXW + M * NH]
        V_a = aux_t[:, AUXW + M * NH:AUXL].bitcast(mm_dt)   # [KD, M*NH*S] fp16
        _w0 = 0
        oh_t = [const.tile([64, L], mm_dt, tag=f"oh{g}", name=f"oh{g}")
                for g in range(NGRP)]
        nc.sync.dma_start(oh_t[0][:], oh_e[:, 0, :])
        oh_rest = const.tile([64, (NGRP - 1) * L], mm_dt, tag="oh_rest")
        nc.sync.dma_start(
            oh_rest[:].rearrange("p (g l) -> p g l", g=NGRP - 1),
            oh_e[:, 1:NGRP, :])

        ones80 = const.tile([1, KD], f32, tag="ones80")
        nc.vector.memset(ones80[:], 1.0)
        # Dummy Exp with no input deps: hoists the ACT table load (~1.3us)
        # off the critical path, overlapping it with the input DMAs.
        warm = work.tile([1, KD], f32, tag="warm", name="warm")
        nc.scalar.activation(warm[:], ones80[:], AF.Exp)

        # ---- tau gather (PE, one matmul: replicated-tau stationary gives
        # the KD-row broadcast) -> e = exp(tau*lam) (ACT) -> Ve (DVE
        # broadcast outer product), per m / half
        ve_t = {}
        for m in range(M):
            taub_ps = ps_g.tile([KD, BLOC], f32, tag="g_ps", name=f"taub{m}")
            nc.tensor.matmul(taub_ps[:], aux[:, m * KD:(m + 1) * KD],
                             aux[:, M * KD + m * BLOC:M * KD + (m + 1) * BLOC])
            for h in range(NH):
                mh = m * NH + h
                et = const.tile([KD, BLOC], f32, tag=f"e{mh}", name=f"e{mh}")
                nc.scalar.activation(et[:], taub_ps[:], AF.Exp,
                                     scale=lam_a[:, mh:mh + 1])
                ve = const.tile([KD, BLOC, S], mm_dt, tag=f"ve{mh}",
                                name=f"ve{mh}")
                v_b = V_a[:, mh * S:(mh + 1) * S] \
                    .unsqueeze(1).broadcast_to((KD, BLOC, S))
                e_b = et[:].unsqueeze(2).broadcast_to((KD, BLOC, S))
                nc.vector.tensor_tensor(ve[:], v_b, e_b, ALU.mult)
                ve_t[mh] = ve

        # ---- per group: P -> ptab -> per-pair gather+copy+DMA
        for g in range(NGRP):
            m, qh = g // 2, g % 2
            P_ps = ps_p.tile([64, KJ], f32, tag="P_ps", name=f"P_ps{g}")
            ptab = work.tile([64, KJ], mm_dt, tag="ptab", name=f"ptab{g}",
                             bufs=NGRP)
            for u in range(2):
                q = 2 * qh + u
                pslice = slice(32 * u, 32 * u + S)
                for h in range(NH):
                    mh = m * NH + h
                    nc.tensor.matmul(
                        P_ps[pslice, :], ve_t[mh][:, q, :],
                        cst[:, _w0 + mh * KJ:_w0 + (mh + 1) * KJ],
                        start=(h == 0), stop=(h == NH - 1))
            if g % 2 == 0:
                nc.vector.tensor_copy(ptab[:], P_ps[:])
            else:
                nc.scalar.activation(ptab[:], P_ps[:], AF.Copy)
            oh_g = oh_t[g] if g == 0 else oh_rest[:, (g - 1) * L:g * L]
            for u in range(2):
                pr = 2 * g + u
                pslice = slice(32 * u, 32 * u + S)
                o_sb = outp.tile([LC, NCHUNK * KJ], f32, tag="o_sb",
                                 name="o_sb")
                for cj in range(2):
                    # [128, 512] = 1 PSUM bank; chunks 2cj,2cj+1 at 0/160
                    g_ps = ps_g.tile([LC, 512], f32, tag="g_ps", name="g_ps")
                    for w in range(2):
                        ci = 2 * cj + w
                        nc.tensor.matmul(
                            g_ps[:, w * KJ:(w + 1) * KJ],
                            oh_g[pslice, ci::NCHUNK],
                            ptab[pslice, :])
                    if (u + cj) % 2 == 0:
                        nc.vector.tensor_copy(
                            o_sb[:, cj * 2 * KJ:(cj + 1) * 2 * KJ],
                            g_ps[:, 0:2 * KJ])
                    else:
                        nc.scalar.activation(
                            o_sb[:, cj * 2 * KJ:(cj + 1) * 2 * KJ],
                            g_ps[:, 0:2 * KJ], AF.Copy)
                nc.sync.dma_start(out_e[pr], o_sb[:])

    nc.compile()
    _GRAPH_CACHE["nc"] = nc
    return nc


def _run(inputs, trace=False):
    from concourse.bass_utils import run_bass_kernel_spmd
    in_maps = _host_prep(**inputs)
    nc = _build_graph()
    res = run_bass_kernel_spmd(nc, in_maps, core_ids=list(range(NCORES)),
                               trace=trace)
    full = np.empty((M, B, L, K, S), np.float32)
    for c in range(NCORES):
        # out[pr, p, (ci, k, s)] holds l = 4*p + ci
        o = res.results[c]["out"].reshape(M, BLOC, LC, NCHUNK, K, S)
        full[:, c * BLOC:(c + 1) * BLOC] = o.reshape(M, BLOC, L, K, S)
    return full, res


def kernel(sequences, rate_indices, tau_kernel, exchangeability_kernel,
           equilibrium_kernel):
    out, _ = _run(dict(sequences=sequences, rate_indices=rate_indices,
                       tau_kernel=tau_kernel,
                       exchangeability_kernel=exchangeability_kernel,
                       equilibrium_kernel=equilibrium_kernel))
    return out
